# revision 16
# baseline (speedup 1.0000x reference)
"""Multi-head causal self-attention (B=4, T=2048, C=1024, H=16) on 8 TRN2 cores.

Sharding: core pair {2b, 2b+1} owns batch b; even core computes heads 0-7,
odd core heads 8-15 (tensor parallel over heads). Each core:
  1. qkvT projection from host-transposed xT (bf16 matmuls, fp32 PSUM),
     emitted incrementally per 512-wide T-chunk
  2. causal attention processed chunk 0 -> 3 in scoresT [Tk, Tq]
     orientation; per k-tile: two heads' score matmuls packed into disjoint
     PE row groups (concurrent via tile_position), one exp per pair on
     ScalarE (scale=1/8), AV^T matmuls with an appended ones column
     producing softmax denominators for free.  Diagonal k-tiles compute
     only the un-masked column range (partial-N) and apply one shared
     [128,128] triangle mask.
  3. softmax: per-head reciprocal into an [8,...] rsum tile at the end of
     each attn call; K=8 selector-matmul broadcast + in-place multiply
     deferred into the next chunk as PE fillers
  4. partial c_proj over local 512 channels (bias/2 via K=1 matmul; v-bias
     and proj bias folded into one per-column constant)
  5. chunked pairwise bf16 ReduceScatter for chunks 0-2 overlapped with
     later chunks; chunk 3 emitted as per-core partials summed on host.
Every qkv/proj chain is injected as a filler into the exp-paced attention
m-loops so the PE never idles long enough to re-throttle (HAM).
Host only shards/casts inputs and concatenates the 8 output slices.
"""

import math
import numpy as np
import ml_dtypes
from contextlib import ExitStack

import concourse.bass as bass
import concourse.tile as tile
from concourse import bacc, mybir
from concourse.bass_utils import run_bass_kernel_spmd

bf16 = ml_dtypes.bfloat16
F32 = mybir.dt.float32
BF16 = mybir.dt.bfloat16
AF = mybir.ActivationFunctionType
ADD = mybir.AluOpType.add

B, T, C, H = 4, 2048, 1024, 16
D = C // H              # 64 head dim
NCORES = 8
HL = H // 2             # 8 heads per core
CL = HL * D             # 512 local channels
PAIRS = [[0, 1], [2, 3], [4, 5], [6, 7]]

_CACHE = {}


def _build():
    nc = bacc.Bacc("TRN2", target_bir_lowering=False, debug=False,
                   num_devices=NCORES)

    xT_d = nc.dram_tensor("xT", [C, T], BF16, kind="ExternalInput").ap()
    wa_d = nc.dram_tensor("wa", [C, 3 * CL], BF16, kind="ExternalInput").ap()
    wp_d = nc.dram_tensor("wp", [CL, C], BF16, kind="ExternalInput").ap()
    bqk_d = nc.dram_tensor("bqk", [128, 8], F32, kind="ExternalInput").ap()
    bye_d = nc.dram_tensor("bye", [1, C], BF16, kind="ExternalInput").ap()
    tri_d = nc.dram_tensor("tri", [128, 1, 128], BF16, kind="ExternalInput").ap()
    sel_d = nc.dram_tensor("sel", [4, 8, 128], BF16, kind="ExternalInput").ap()
    y_d = nc.dram_tensor("y", [3, T // 8, C], BF16, kind="ExternalOutput").ap()
    y3_d = nc.dram_tensor("y3", [512, C], BF16, kind="ExternalOutput").ap()

    with tile.TileContext(nc) as tc, ExitStack() as ctx:
        cst = ctx.enter_context(tc.tile_pool(name="cst", bufs=1))
        work = ctx.enter_context(tc.tile_pool(name="work", bufs=6))
        ysb_p = ctx.enter_context(tc.tile_pool(name="ysb", bufs=2))
        rsl_p = ctx.enter_context(tc.tile_pool(name="rsl", bufs=4))
        ps_mm = ctx.enter_context(tc.tile_pool(name="psmm", bufs=2, space="PSUM"))
        ps_s = ctx.enter_context(tc.tile_pool(name="pss", bufs=2, space="PSUM"))
        ps_av = ctx.enter_context(tc.tile_pool(name="psav", bufs=2, space="PSUM"))
        dram = ctx.enter_context(tc.tile_pool(name="dram", bufs=1, space="DRAM"))

        # ---- persistent SBUF tensors ----
        xT = cst.tile([128, 8, T], BF16)        # x^T  (C on partitions)
        wa = cst.tile([128, 8, 3 * CL], BF16)   # W_attn col-slice [q|k|v]
        wp = cst.tile([128, 4, C], BF16)        # W_proj local rows
        bqk = cst.tile([128, 8], F32)           # q/k biases per qT/kT tile
        bye = cst.tile([1, C], BF16)            # (b_proj + bv@W_proj)/2
        tri = cst.tile([128, 1, 128], BF16)     # shared diagonal triangle mask
        ones = cst.tile([1, 128], BF16)
        sel = cst.tile([8, 4, 128], BF16)       # per-j K=8 broadcast selectors
        qT = cst.tile([128, 4, T], BF16)
        kT = cst.tile([128, 4, T], BF16)
        vaug = cst.tile([128, 16, HL, D + 1], BF16)  # v rows + ones col
        avT = cst.tile([128, 4, T], BF16)       # attn-out^T (raw, then normed)
        sums = cst.tile([8, 4, 512], F32)       # softmax denominators [h, c, tq]
        rsum = cst.tile([8, 4, 512], BF16)      # their reciprocals

        xT_r = xT_d.rearrange("(ko p) t -> p ko t", p=128)
        wa_r = wa_d.rearrange("(ko p) n -> p ko n", p=128)

        # ---- input DMAs.  dma_start blocks the issuing engine's queue for
        # roughly the transfer time, so bulk loads go on sync + gpsimd
        # (idle until mid-kernel); scalar's ring starts earliest, so it
        # carries the small first-needed pieces, then stays free for exp.
        nc.scalar.dma_start(wa[:, :, 0:128], wa_r[:, :, 0:128])        # q j0
        nc.scalar.dma_start(wa[:, :, CL:CL + 128], wa_r[:, :, CL:CL + 128])
        nc.sync.dma_start(xT[:, :, 0:512], xT_r[:, :, 0:512])
        nc.gpsimd.dma_start(wa[:, :, 2 * CL:3 * CL], wa_r[:, :, 2 * CL:3 * CL])
        nc.gpsimd.dma_start(wa[:, :, 128:CL], wa_r[:, :, 128:CL])      # q j1-3
        nc.gpsimd.dma_start(wa[:, :, CL + 128:2 * CL],
                            wa_r[:, :, CL + 128:2 * CL])               # k j1-3
        nc.sync.dma_start(xT[:, :, 512:1024], xT_r[:, :, 512:1024])
        nc.sync.dma_start(xT[:, :, 1024:1536], xT_r[:, :, 1024:1536])
        nc.sync.dma_start(xT[:, :, 1536:2048], xT_r[:, :, 1536:2048])
        nc.gpsimd.dma_start(wp[:], wp_d.rearrange("(ko p) n -> p ko n", p=128))
        nc.vector.memset(ones[:], 1.0)
        nc.vector.memset(vaug[:, :, :, D], 1.0)
        nc.vector.memset(rsum[:], 0.0)
        nc.scalar.dma_start(bqk[:], bqk_d)
        nc.scalar.dma_start(tri[:], tri_d)
        nc.scalar.dma_start(sel[:], sel_d.rearrange("j k p -> k j p"))
        nc.scalar.dma_start(bye[:], bye_d)

        rs_in = dram.tile([3, 512, C], BF16)
        rs_out = dram.tile([3, 256, C], BF16)

        # ---- QKV projection chains ----
        # qT/kT tile j holds heads {2j, 2j+1}.  qT = W_q^T @ x^T.
        def qk_chain(qk, j, ch):
            dst = qT if qk == 0 else kT
            ps = ps_mm.tile([128, 512], F32, tag="mm")
            for ko in range(8):
                nc.tensor.matmul(
                    ps[:],
                    lhsT=wa[:, ko, qk * CL + 128 * j:qk * CL + 128 * (j + 1)],
                    rhs=xT[:, ko, 512 * ch:512 * (ch + 1)],
                    start=(ko == 0), stop=(ko == 7))
            nc.vector.tensor_tensor(
                dst[:, j, 512 * ch:512 * (ch + 1)], ps[:],
                bqk[:, 4 * qk + j:4 * qk + j + 1].to_broadcast((128, 512)),
                ADD)

        # v in natural [T, c_local] layout, interleaved with ones columns
        def v_chain(m):
            ps = ps_mm.tile([128, 512], F32, tag="mm")
            for ko in range(8):
                nc.tensor.matmul(
                    ps[:],
                    lhsT=xT[:, ko, 128 * m:128 * (m + 1)],
                    rhs=wa[:, ko, 2 * CL:3 * CL],
                    start=(ko == 0), stop=(ko == 7))
            nc.vector.tensor_copy(
                vaug[:, m, :, 0:D],
                ps[:].rearrange("p (h d) -> p h d", d=D))

        def recip_piece(c, i):
            """Quarter-width reciprocal: reciprocal cost scales with the
            per-partition element count, so [8,128] quarters are cheap and
            keep the in-order DVE stream responsive."""
            with nc.allow_low_precision(reason="softmax reciprocal in bf16"):
                nc.vector.reciprocal(rsum[:, c, 128 * i:128 * (i + 1)],
                                     sums[:, c, 128 * i:128 * (i + 1)])

        def norm_mult(c, j):
            """Broadcast rsum rows {2j,2j+1} over 64 partitions each and
            normalize avT in place (deferred from attn(j, c))."""
            pbc = ps_mm.tile([128, 512], F32, tag="mm")
            nc.tensor.matmul(pbc[:], lhsT=sel[:, j, :], rhs=rsum[:, c, :],
                             start=True, stop=True)
            nc.vector.tensor_mul(
                avT[:, j, 512 * c:512 * (c + 1)],
                avT[:, j, 512 * c:512 * (c + 1)], pbc[:])

        ysb_tiles = {}

        def proj_chain(mt, n):
            """One 512-col n-chunk of c_proj for T-tile mt (+ DMA on n=1)."""
            q4 = mt // 4
            if n == 0:
                ysb_tiles[mt] = ysb_p.tile([128, C], BF16, tag="y",
                                           name=f"ysb{mt}")
            ysb = ysb_tiles[mt]
            ps = ps_mm.tile([128, 512], F32, tag="mm")
            nc.tensor.matmul(ps[:], lhsT=ones[0:1, :],
                             rhs=bye[0:1, 512 * n:512 * (n + 1)],
                             start=True, stop=False)
            for j2 in range(4):
                nc.tensor.matmul(
                    ps[:],
                    lhsT=avT[:, j2, 128 * mt:128 * (mt + 1)],
                    rhs=wp[:, j2, 512 * n:512 * (n + 1)],
                    start=False, stop=(j2 == 3))
            nc.vector.tensor_copy(ysb[:, 512 * n:512 * (n + 1)], ps[:])
            if n == 1:
                del ysb_tiles[mt]
                if q4 == 3:
                    nc.sync.dma_start(
                        y3_d[128 * (mt % 4):128 * (mt % 4 + 1), :], ysb[:])
                else:
                    nc.sync.dma_start(
                        rs_in[q4, 128 * (mt % 4):128 * (mt % 4 + 1), :], ysb[:])

        def rs_chunk(q4):
            nc.gpsimd.collective_compute(
                "ReduceScatter", ADD, replica_groups=PAIRS,
                ins=[rs_in[q4]], outs=[rs_out[q4]])
            # y DMA waits ~20us for the collective; keep it off the sync
            # queue so later sums/proj DMAs aren't head-of-line blocked
            nc.gpsimd.dma_start(y_d[q4], rs_out[q4])

        def attn(j, c, fillers):
            """Head pair {2j, 2j+1}, Tq chunk c.  fillers: closures emitted
            at evenly spread m positions to keep the PE fed while the m-loop
            is paced by ScalarE exp."""
            fillers = list(fillers)
            nf = len(fillers)
            ntk = 4 * (c + 1)
            pos = [(k * ntk) // nf for k in range(nf)] if nf else []
            fi = 0
            pavA = ps_av.tile([D + 1, 512], F32, tag="av")
            pavB = ps_av.tile([D + 1, 512], F32, tag="av")
            for m in range(ntk):
                while fi < nf and pos[fi] <= m:
                    fillers[fi]()
                    fi += 1
                s = m - 4 * c           # >=0: diagonal tile index
                o = 128 * s if s > 0 else 0
                pss = ps_s.tile([128, 2, 512], F32, tag="s")
                for hh in range(2):
                    ro = hh * 64
                    nc.tensor.matmul(
                        pss[:, hh, o:512],
                        lhsT=kT[ro:ro + 64, j, 128 * m:128 * (m + 1)],
                        rhs=qT[ro:ro + 64, j, 512 * c + o:512 * (c + 1)],
                        start=True, stop=True)
                ex = work.tile([128, 2, 512], BF16, tag="expT")
                nc.scalar.activation(ex[:, :, o:512], pss[:, :, o:512],
                                     AF.Exp, scale=1.0 / math.sqrt(D))
                if s >= 0:
                    # triangle mask on the otherwise-idle GpSimd engine so
                    # the in-order DVE queue never gates the AV matmuls
                    nc.gpsimd.tensor_mul(
                        ex[:, :, o:o + 128], ex[:, :, o:o + 128],
                        tri[:].to_broadcast((128, 2, 128)))
                for hh in range(2):
                    nc.tensor.matmul(
                        (pavA if hh == 0 else pavB)[:, o:512],
                        lhsT=vaug[:, m, 2 * j + hh, :],
                        rhs=ex[:, hh, o:512],
                        start=(m == 0), stop=(m == ntk - 1))
            while fi < nf:
                fillers[fi]()
                fi += 1
            # per-head denominator extraction + raw avT evacuation; the
            # reciprocal and broadcast-multiply are deferred into the next
            # chunk (reciprocal on one partition is ~5x slower per element,
            # so it runs later as [8,128] quarters).
            for hh in range(2):
                pav = pavA if hh == 0 else pavB
                ro = hh * 64
                stg = rsl_p.tile([1, 512], F32, tag="rs")
                nc.vector.tensor_copy(stg[:], pav[D:D + 1, :])
                nc.sync.dma_start(sums[2 * j + hh:2 * j + hh + 1, c, :],
                                  stg[:])
                nc.vector.tensor_copy(
                    avT[ro:ro + 64, j, 512 * c:512 * (c + 1)], pav[0:D, :])

        # ---- filler schedules ----
        def f_qk(qk, j, ch):
            return lambda: qk_chain(qk, j, ch)

        def f_v(m):
            return lambda: v_chain(m)

        def f_nm(c, j):
            return lambda: norm_mult(c, j)

        def f_rp(c, i):
            return lambda: recip_piece(c, i)

        def f_proj(mt, n):
            return lambda: proj_chain(mt, n)

        def f_rs(q4):
            return lambda: rs_chunk(q4)

        fill = {(c, j): [] for c in range(4) for j in range(4)}
        for c in range(4):
            for j in range(3):
                fill[(c, j)] += [f_qk(0, j + 1, c), f_qk(1, j + 1, c)]
        for c in range(3):
            fill[(c, 3)] += [f_qk(0, 0, c + 1), f_qk(1, 0, c + 1)]
            fill[(c, 3)] += [f_v(4 * (c + 1) + i) for i in range(4)]
        # chunk c-1 normalization + proj front-loaded into calls (c, 0..1)
        # so the ReduceScatter can trigger by call (c, 2) and its ~20-30us
        # collective + output DMA finish well before they are waited on
        for c in range(1, 4):
            fill[(c, 0)] += [f_rp(c - 1, i) for i in range(4)]
            fill[(c, 0)] += [f_nm(c - 1, jj) for jj in range(4)]
            pcs = [f_proj(mt, n)
                   for mt in range(4 * (c - 1), 4 * (c - 1) + 4)
                   for n in (0, 1)]
            fill[(c, 0)] += pcs[0:4]
            fill[(c, 1)] += pcs[4:8]
            fill[(c, 2)].append(f_rs(c - 1))

        # ---- emission ----
        qk_chain(0, 0, 0)
        qk_chain(1, 0, 0)
        for m in range(4):
            v_chain(m)
        for c in range(4):
            for j in range(4):
                attn(j, c, fill[(c, j)])
        # tail: chunk 3 normalization (proj tiles contract ALL j pairs, so
        # every norm_mult must precede the first proj chain)
        for i in range(4):
            recip_piece(3, i)
        for jj in range(4):
            norm_mult(3, jj)
        for mt in range(12, 16):
            proj_chain(mt, 0)
            proj_chain(mt, 1)

    nc.compile()
    return nc


def _prep_inputs(x, W_attn, b_attn, W_proj, b_proj):
    x = np.asarray(x, dtype=np.float32)
    W_attn = np.asarray(W_attn, dtype=np.float32)
    b_attn = np.asarray(b_attn, dtype=np.float32)
    W_proj = np.asarray(W_proj, dtype=np.float32)
    b_proj = np.asarray(b_proj, dtype=np.float32)

    bv = b_attn[2 * C:3 * C]
    bye_full = (b_proj + bv @ W_proj) * 0.5
    bye = bye_full[None, :].astype(bf16)

    # shared triangle mask for diagonal k-tiles: keep iff p <= f
    tri = (np.arange(128)[:, None, None] <= np.arange(128)[None, None, :])
    tri = tri.astype(np.float32).astype(bf16)

    # K=8 broadcast selectors: out partition p gets rsum row 2j + (p >= 64)
    sel = np.zeros((4, 8, 128), np.float32)
    for j in range(4):
        sel[j, 2 * j, 0:64] = 1.0
        sel[j, 2 * j + 1, 64:128] = 1.0
    sel = sel.astype(bf16)

    in_maps = []
    for c in range(NCORES):
        b, r = c // 2, c % 2
        xT = np.ascontiguousarray(x[b].T).astype(bf16)
        qs, ks, vs = CL * r, C + CL * r, 2 * C + CL * r
        wa = np.concatenate([W_attn[:, qs:qs + CL], W_attn[:, ks:ks + CL],
                             W_attn[:, vs:vs + CL]], axis=1).astype(bf16)
        wp = W_proj[CL * r:CL * (r + 1), :].astype(bf16)
        bqk = np.empty((128, 8), np.float32)
        for j in range(4):
            bqk[:, j] = b_attn[qs + 128 * j:qs + 128 * (j + 1)]
            bqk[:, 4 + j] = b_attn[ks + 128 * j:ks + 128 * (j + 1)]
        in_maps.append({"xT": np.asarray(xT), "wa": np.asarray(wa),
                        "wp": np.asarray(wp), "bqk": bqk,
                        "bye": np.asarray(bye), "tri": np.asarray(tri),
                        "sel": np.asarray(sel)})
    return in_maps


def kernel(x, W_attn, b_attn, W_proj, b_proj, _trace=False, _result=[None]):
    if "nc" not in _CACHE:
        _CACHE["nc"] = _build()
    nc = _CACHE["nc"]
    in_maps = _prep_inputs(x, W_attn, b_attn, W_proj, b_proj)
    res = run_bass_kernel_spmd(nc, in_maps, list(range(NCORES)), trace=_trace)
    _result[0] = res
    out = np.empty((B, T, C), np.float32)
    for c in range(NCORES):
        b, r = c // 2, c % 2
        yc = res.results[c]["y"].astype(np.float32)  # [3, 256, C] chunks 0-2
        for q4 in range(3):
            out[b, 512 * q4 + 256 * r:512 * q4 + 256 * (r + 1), :] = yc[q4]
    for b in range(B):
        out[b, 1536:2048, :] = (res.results[2 * b]["y3"].astype(np.float32)
                                + res.results[2 * b + 1]["y3"].astype(np.float32))
    return out
